# revision 1
# baseline (speedup 1.0000x reference)
"""Multi-head attention (B=4, S=2048, D=1024, H=16, Hd=64) on 8 NeuronCores.

Sharding: tensor-parallel over heads. Core c owns heads {2c, 2c+1}, i.e. a
128-column slice of Wq/Wk/Wv and the matching 128-row slice of Wo. Each core
computes a full-shape partial output (its heads' contribution through the out
projection); the host sums the 8 partials.

Host-side prep is layout/dtype only (transpose to [D, B*S], cast bf16, slice
weights) plus two exact bias identities:
  * softmax rows sum to 1, so bv contributes exactly (bv @ Wo) to every
    output row -> added on host.
  * bk shifts all scores of a row equally and cancels in softmax; it is
    still applied on-device (free during K-projection eviction).

Device algorithm per core (all matmuls bf16 with f32 PSUM accumulation):
  1. QT = (Wq_c^T q^T + bq_c), KT likewise  -> SBUF [128=d', 8192=s] bf16
     V   = v @ Wv_c                          -> SBUF [s, d'] bf16, stored as
     64 chunks of [128, 130]: cols 0:64 head0-V, 64:66 ones, 66:130 head1-V.
  2. Per (batch, q-slice of 512), interleaved over the 2 heads:
     scores^T tile = K_h Q_h^T (K=64 matmul, 2-head row-tiled on the PE),
     P^T = exp(scores^T / 8) on ScalarE (no max subtraction needed: scores
     are ~N(0,1), nowhere near f32 overflow),
     O^T accumulation: lhsT = [V_h | ones] so the PSUM picks up both the
     attention numerator rows and the softmax row-sum row in one matmul.
     Normalize: reciprocal of the row-sum row, GPSIMD partition-broadcast,
     DVE multiply -> OT SBUF [128=d', 8192=s] bf16.
  3. out_partial = OT^T @ Wo_c per s-tile -> DRAM f32.
"""

import os
from contextlib import ExitStack

import numpy as np
import ml_dtypes

import concourse.bass as bass
import concourse.mybir as mybir
import concourse.tile as tile
from concourse import bacc, library_config
from concourse.bass_utils import run_bass_kernel_spmd

B, S, D, H, HD = 4, 2048, 1024, 16, 64
BS = B * S                     # 8192 flattened tokens
NCORES = 8
HPC = H // NCORES              # 2 heads per core
DC = HPC * HD                  # 128-wide weight slice per core

F32 = mybir.dt.float32
BF16 = mybir.dt.bfloat16
EXP = mybir.ActivationFunctionType.Exp

_BUILT = None                  # (nc, tmpdir) cache — compile once per process
LAST_EXEC_NS = None
LAST_RESULTS = None


def _build_program():
    nc = bacc.Bacc("TRN2", target_bir_lowering=False, debug=False,
                   num_devices=NCORES)

    qT_d = nc.dram_tensor("qT", [D, BS], BF16, kind="ExternalInput").ap()
    kT_d = nc.dram_tensor("kT", [D, BS], BF16, kind="ExternalInput").ap()
    vT_d = nc.dram_tensor("vT", [D, BS], BF16, kind="ExternalInput").ap()
    wq_d = nc.dram_tensor("wq", [D, DC], BF16, kind="ExternalInput").ap()
    wk_d = nc.dram_tensor("wk", [D, DC], BF16, kind="ExternalInput").ap()
    wv_d = nc.dram_tensor("wv", [D, DC], BF16, kind="ExternalInput").ap()
    wo_d = nc.dram_tensor("wo", [DC, D], BF16, kind="ExternalInput").ap()
    bq_d = nc.dram_tensor("bq", [DC, 1], F32, kind="ExternalInput").ap()
    bk_d = nc.dram_tensor("bk", [DC, 1], F32, kind="ExternalInput").ap()
    out_d = nc.dram_tensor("out", [BS, D], F32, kind="ExternalOutput").ap()

    with tile.TileContext(nc) as tc, ExitStack() as ctx:
        const = ctx.enter_context(tc.tile_pool(name="const", bufs=1))
        persist = ctx.enter_context(tc.tile_pool(name="persist", bufs=1))
        stage = ctx.enter_context(tc.tile_pool(name="stage", bufs=3))
        ptpool = ctx.enter_context(tc.tile_pool(name="ptpool", bufs=6))
        npool = ctx.enter_context(tc.tile_pool(name="npool", bufs=3))
        ostage = ctx.enter_context(tc.tile_pool(name="ostage", bufs=3))
        # PSUM: "big" 3 slots x 2 banks (scores + all transient accумulators)
        #       "pop" 2 slots x 1 bank (the two per-head O^T accumulators)
        big = ctx.enter_context(tc.tile_pool(name="big", bufs=3, space="PSUM"))
        pop = ctx.enter_context(tc.tile_pool(name="pop", bufs=2, space="PSUM"))

        # ---- persistent SBUF state -------------------------------------
        QT = persist.tile([128, BS], BF16)          # [d' , s]
        KT = persist.tile([128, BS], BF16)
        OT = persist.tile([128, BS], BF16)
        # V extended, per 128-token chunk (free layout [2, 132], abs width 264):
        #   abs cols 0:64    = V_h0          (h0 lhsT = abs 0:65, rsum row 64)
        #   abs col  64      = ones
        #   abs col  68      = ones          (h1 lhsT = abs 68:196, rsum row 0)
        #   abs cols 132:196 = V_h1          (-> h1 lhsT rows 64:128)
        #   everything else zero (h1 lhsT rows 1:64 are garbage, never read)
        VE = persist.tile([128, 64, 2, 132], BF16)

        # ---- constants --------------------------------------------------
        wq_sb = const.tile([128, 8, DC], BF16)
        wk_sb = const.tile([128, 8, DC], BF16)
        wv_sb = const.tile([128, 8, DC], BF16)
        wo_sb = const.tile([128, D], BF16)
        bq_sb = const.tile([128, 1], F32)
        bk_sb = const.tile([128, 1], F32)
        ones_sb = const.tile([128, 64], F32)
        nc.vector.memset(ones_sb[:], 1.0)
        nc.sync.dma_start(wq_sb[:], wq_d.rearrange("(c p) d -> p c d", p=128))
        nc.sync.dma_start(wk_sb[:], wk_d.rearrange("(c p) d -> p c d", p=128))
        nc.sync.dma_start(wv_sb[:], wv_d.rearrange("(c p) d -> p c d", p=128))
        nc.sync.dma_start(wo_sb[:], wo_d)
        nc.sync.dma_start(bq_sb[:], bq_d)
        nc.sync.dma_start(bk_sb[:], bk_d)
        nc.vector.memset(VE[:], 0.0)
        nc.vector.memset(VE[:, :, 0, 64:65], 1.0)
        nc.vector.memset(VE[:, :, 0, 68:69], 1.0)

        # ---- phase 1: projections --------------------------------------
        for srcT, w_sb, b_sb, dstT in (
            (qT_d, wq_sb, bq_sb, QT),
            (kT_d, wk_sb, bk_sb, KT),
        ):
            for ss in range(16):                    # s-slices of 512
                xt = stage.tile([128, 8, 512], BF16, tag="xT")
                nc.sync.dma_start(
                    xt[:],
                    srcT.rearrange("(c p) s -> p c s", p=128)[
                        :, :, ss * 512:(ss + 1) * 512],
                )
                ps = big.tile([128, 512], F32, tag="sc", name="psqk")
                for c in range(8):
                    nc.tensor.matmul(ps[:], lhsT=w_sb[:, c], rhs=xt[:, c],
                                     start=(c == 0), stop=(c == 7))
                nc.vector.tensor_scalar_add(
                    dstT[:, ss * 512:(ss + 1) * 512], ps[:], b_sb[:])

        for ss in range(16):
            vt = stage.tile([128, 8, 512], BF16, tag="xT")
            nc.sync.dma_start(
                vt[:],
                vT_d.rearrange("(c p) s -> p c s", p=128)[
                    :, :, ss * 512:(ss + 1) * 512],
            )
            for st in range(4):                     # s-tiles of 128
                chunk = ss * 4 + st
                ps = big.tile([128, 512], F32, tag="sc", name="psv")
                for c in range(8):
                    nc.tensor.matmul(
                        ps[:, 0:DC],
                        lhsT=vt[:, c, st * 128:(st + 1) * 128],
                        rhs=wv_sb[:, c],
                        start=(c == 0), stop=(c == 7))
                # h0 cols 0:64 -> VE[..,0,0:64]; h1 cols 64:128 -> VE[..,1,0:64]
                dst = VE[:, chunk, :, 0:64]
                src = ps[:, 0:DC].rearrange("p (a x) -> p a x", a=2)
                nc.vector.tensor_copy(dst, src)

        # ---- phase 2+3: attention, then out-proj, per batch -------------
        for b in range(B):
            for qs in range(4):                     # q-slices of 512
                q0 = b * S + qs * 512
                po = [pop.tile([128, 512], F32, tag="po", name=f"po{h}")
                      for h in range(HPC)]
                def emit_pv(kg, pt):
                    # O^T accumulation for k-group kg, both heads
                    for h in range(HPC):
                        for kt2 in range(2):
                            chunk = b * 16 + kg * 2 + kt2
                            first = (kg == 0 and kt2 == 0)
                            last = (kg == 7 and kt2 == 1)
                            ve_flat = VE[:, chunk, :, :].rearrange(
                                "p a x -> p (a x)")
                            if h == 0:
                                # rows 0:64 = O^T_h0, row 64 = rowsum_h0
                                nc.tensor.matmul(
                                    po[0][0:65, :],
                                    lhsT=ve_flat[:, 0:65],
                                    rhs=pt[0][:, kt2],
                                    start=first, stop=last)
                            else:
                                # abs cols 68:196: row 0 = rowsum_h1 (ones
                                # at abs 68), rows 64:128 = O^T_h1 (V_h1)
                                nc.tensor.matmul(
                                    po[1][:, :],
                                    lhsT=ve_flat[:, 68:196],
                                    rhs=pt[1][:, kt2],
                                    start=first, stop=last)

                # Software pipeline: PV for k-group kg-1 is emitted between
                # QK(kg) and QK(kg+1), so the PE never sits waiting on the
                # ScalarE exp of the k-group it just produced.
                prev_pt = None
                for kg in range(8):                 # k-groups of 2x128
                    pt = []
                    for h in range(HPC):
                        psc = big.tile([128, 2, 512], F32, tag="sc", name="psc")
                        for kt2 in range(2):
                            k0 = b * S + (kg * 2 + kt2) * 128
                            nc.tensor.matmul(
                                psc[:, kt2],
                                lhsT=KT[64 * h:64 * h + 64, k0:k0 + 128],
                                rhs=QT[64 * h:64 * h + 64, q0:q0 + 512],
                                start=True, stop=True)
                        p = ptpool.tile([128, 2, 512], BF16, tag="pt")
                        nc.scalar.activation(
                            p.rearrange("p a x -> p (a x)"),
                            psc.rearrange("p a x -> p (a x)"),
                            EXP, scale=0.125)
                        pt.append(p)
                    if prev_pt is not None:
                        emit_pv(kg - 1, prev_pt)
                    prev_pt = pt
                emit_pv(7, prev_pt)
                # normalize -> OT. The per-q reciprocal rowsum lives on one
                # partition; replicate it across the head's 64 partitions
                # with a K=1 PE matmul (ones column x reciprocal row).
                for h in range(HPC):
                    rs = 64 if h == 0 else 0        # rowsum partition
                    d0 = 64 * h                     # head's partition base
                    rr = npool.tile([128, 512], F32, tag="rr")
                    nc.vector.reciprocal(rr[rs:rs + 1, :], po[h][rs:rs + 1, :])
                    bcp = big.tile([128, 512], F32, tag="sc", name="bcp")
                    nc.tensor.matmul(
                        bcp[d0:d0 + 64, :],
                        lhsT=ones_sb[rs:rs + 1, :],
                        rhs=rr[rs:rs + 1, :],
                        start=True, stop=True)
                    bc = npool.tile([128, 512], F32, tag="bc")
                    nc.vector.tensor_copy(bc[d0:d0 + 64, :], bcp[d0:d0 + 64, :])
                    osrc = po[h][0:64, :] if h == 0 else po[h][64:128, :]
                    nc.vector.tensor_mul(
                        OT[d0:d0 + 64, q0:q0 + 512], osrc, bc[d0:d0 + 64, :])

            # out-projection for this batch's s-range
            for st in range(16):
                s0 = b * S + st * 128
                osb = ostage.tile([128, D], F32, tag="osb")
                for ns in range(2):
                    ps3 = big.tile([128, 512], F32, tag="sc", name="ps3")
                    nc.tensor.matmul(
                        ps3[:],
                        lhsT=OT[:, s0:s0 + 128],
                        rhs=wo_sb[:, ns * 512:(ns + 1) * 512],
                        start=True, stop=True)
                    if ns == 0:
                        nc.vector.tensor_copy(osb[:, 0:512], ps3[:])
                    else:
                        nc.scalar.copy(osb[:, 512:1024], ps3[:])
                nc.sync.dma_start(out_d[s0:s0 + 128, :], osb[:])

    nc.compile()
    return nc


def _get_program():
    global _BUILT
    if _BUILT is None:
        _BUILT = _build_program()
    return _BUILT


def kernel(q, k, v, Wq, bq, Wk, bk, Wv, bv, Wo, bo, trace=None):
    global LAST_EXEC_NS, LAST_RESULTS
    if trace is None:
        trace = os.environ.get("KERNEL_TRACE", "0") == "1"
    bf16 = ml_dtypes.bfloat16

    q2 = np.asarray(q, np.float32).reshape(BS, D)
    k2 = np.asarray(k, np.float32).reshape(BS, D)
    v2 = np.asarray(v, np.float32).reshape(BS, D)
    qT = np.ascontiguousarray(q2.T).astype(bf16)
    kT = np.ascontiguousarray(k2.T).astype(bf16)
    vT = np.ascontiguousarray(v2.T).astype(bf16)

    Wq = np.asarray(Wq, np.float32)
    Wk = np.asarray(Wk, np.float32)
    Wv = np.asarray(Wv, np.float32)
    Wo = np.asarray(Wo, np.float32)
    bq = np.asarray(bq, np.float32)
    bk = np.asarray(bk, np.float32)
    bv = np.asarray(bv, np.float32)
    bo = np.asarray(bo, np.float32)

    in_maps = []
    for c in range(NCORES):
        sl = slice(c * DC, (c + 1) * DC)
        in_maps.append({
            "qT": qT, "kT": kT, "vT": vT,
            "wq": np.ascontiguousarray(Wq[:, sl]).astype(bf16),
            "wk": np.ascontiguousarray(Wk[:, sl]).astype(bf16),
            "wv": np.ascontiguousarray(Wv[:, sl]).astype(bf16),
            "wo": np.ascontiguousarray(Wo[sl, :]).astype(bf16),
            "bq": np.ascontiguousarray(bq[sl]).reshape(DC, 1),
            "bk": np.ascontiguousarray(bk[sl]).reshape(DC, 1),
        })

    nc = _get_program()
    res = run_bass_kernel_spmd(nc, in_maps, list(range(NCORES)), trace=trace)
    LAST_EXEC_NS = res.exec_time_ns
    LAST_RESULTS = res

    out = np.zeros((BS, D), np.float32)
    for c in range(NCORES):
        out += np.asarray(res.results[c]["out"], np.float32)
    out += bv.astype(np.float32) @ Wo + bo          # exact bias identities
    return out.reshape(B, S, D)



# revision 8
# speedup vs baseline: 1.2539x; 1.2539x over previous
"""Multi-head attention (B=4, S=2048, D=1024, H=16, Hd=64) on 8 NeuronCores.

Sharding: tensor-parallel over heads. Core c owns heads {2c, 2c+1}, i.e. a
128-column slice of Wq/Wk/Wv and the matching 128-row slice of Wo. Each core
computes a full-shape partial output (its heads' contribution through the out
projection); the host sums the 8 partials (f32) plus the exact bias identities
(softmax rows sum to 1 -> bv@Wo + bo added on host; bk cancels in softmax but
is still applied on-device for free).

v2 structure (vs v1): the whole kernel is a single software pipeline over
batches, engineered to keep the PE array continuously busy so the HAM clock
gate stays at 2.4 GHz (v1 ran most matmuls at the cold 1.2 GHz rate):

  * per-batch projection -> attention -> out-projection, with projection and
    out-projection matmuls of neighbouring batches interleaved as "filler"
    units inside the attention chunk loop (PE never idles while ACT does exp).
  * scores for the two heads are issued as a row-tiled pair (h0 rows 0:64,
    h1 rows 64:128 via tile_position auto-derivation) into separate PSUM banks
    of one [128, 2, 512] tile -> they execute concurrently on the PE, and one
    ACT exp (N=1024) covers both heads.
  * softmax normalization: DVE reciprocal_approx_fast (0.7us vs 3.3us for the
    iterative reciprocal), then a col-tiled concurrent pair of K=1 ones-matmul
    broadcasts, one PSUM->SBUF copy, two DVE multiplies.
  * out-projection eviction entirely on DVE (v1 put half on ACT, the exp
    engine), output written bf16 (halves write traffic; host sums in f32).

Device algorithm per core (all matmuls bf16, f32 PSUM):
  1. QT/KT = Wc^T x^T + b  -> SBUF [128=d', 8192=s] bf16 (h0 rows 0:64,
     h1 rows 64:128); V -> SBUF [token, d'] chunks with ones columns for the
     softmax row-sum rows (VE layout [128, chunk, 2, 132]).
  2. Per (batch, q-slice of 512): 16 k-chunks of 128: scores^T pair ->
     exp -> P^T; O^T accumulated per head via [V_h | ones] lhsT (row-sum row
     rides along). Normalize with reciprocal + PE broadcast -> OT bf16.
  3. out_partial = OT^T @ Wo per s-tile -> DRAM bf16.
"""

import os
from contextlib import ExitStack

import numpy as np
import ml_dtypes

import concourse.bass as bass
import concourse.mybir as mybir
import concourse.tile as tile
from concourse import bacc, library_config
from concourse.bass_utils import run_bass_kernel_spmd

B, S, D, H, HD = 4, 2048, 1024, 16, 64
BS = B * S                     # 8192 flattened tokens
NCORES = 8
HPC = H // NCORES              # 2 heads per core
DC = HPC * HD                  # 128-wide weight slice per core

F32 = mybir.dt.float32
BF16 = mybir.dt.bfloat16
EXP = mybir.ActivationFunctionType.Exp

_BUILT = None
LAST_EXEC_NS = None
LAST_RESULTS = None


def _build_program():
    nc = bacc.Bacc("TRN2", target_bir_lowering=False, debug=False,
                   num_devices=NCORES)

    qT_d = nc.dram_tensor("qT", [D, BS], BF16, kind="ExternalInput").ap()
    kT_d = nc.dram_tensor("kT", [D, BS], BF16, kind="ExternalInput").ap()
    vT_d = nc.dram_tensor("vT", [D, BS], BF16, kind="ExternalInput").ap()
    wq_d = nc.dram_tensor("wq", [D, DC], BF16, kind="ExternalInput").ap()
    wk_d = nc.dram_tensor("wk", [D, DC], BF16, kind="ExternalInput").ap()
    wv_d = nc.dram_tensor("wv", [D, DC], BF16, kind="ExternalInput").ap()
    wo_d = nc.dram_tensor("wo", [DC, D], BF16, kind="ExternalInput").ap()
    bq_d = nc.dram_tensor("bq", [DC, 1], F32, kind="ExternalInput").ap()
    bk_d = nc.dram_tensor("bk", [DC, 1], F32, kind="ExternalInput").ap()
    out_d = nc.dram_tensor("out", [BS, D], BF16, kind="ExternalOutput").ap()

    with tile.TileContext(nc) as tc, ExitStack() as ctx:
        const = ctx.enter_context(tc.tile_pool(name="const", bufs=1))
        persist = ctx.enter_context(tc.tile_pool(name="persist", bufs=1))
        stage = ctx.enter_context(tc.tile_pool(name="stage", bufs=3))
        ptpool = ctx.enter_context(tc.tile_pool(name="ptpool", bufs=4))
        npool = ctx.enter_context(tc.tile_pool(name="npool", bufs=4))
        ostage = ctx.enter_context(tc.tile_pool(name="ostage", bufs=3))
        # PSUM: psc 2 slots x 2 banks (scores double-buffer)
        #       acc 2 slots x 1 bank (proj accum / out-proj / bcast)
        #       pop 2 slots x 1 bank (per-head O^T accumulators)
        pscp = ctx.enter_context(tc.tile_pool(name="pscp", bufs=2, space="PSUM"))
        accp = ctx.enter_context(tc.tile_pool(name="accp", bufs=2, space="PSUM"))
        pop = ctx.enter_context(tc.tile_pool(name="pop", bufs=2, space="PSUM"))

        # ---- persistent SBUF state -------------------------------------
        QT = persist.tile([128, BS], BF16)          # [d', s]
        KT = persist.tile([128, BS], BF16)
        OT = persist.tile([128, BS], BF16)
        # V extended, per 128-token chunk (free layout [2, 132], abs width 264):
        #   abs cols 0:64    = V_h0          (h0 lhsT = abs 0:65, rsum row 64)
        #   abs col  64      = ones
        #   abs col  68      = ones          (h1 lhsT = abs 68:196, rsum row 0)
        #   abs cols 132:196 = V_h1          (-> h1 lhsT rows 64:128)
        VE = persist.tile([128, 64, 2, 132], BF16)

        # ---- constants --------------------------------------------------
        wq_sb = const.tile([128, 8, DC], BF16)
        wk_sb = const.tile([128, 8, DC], BF16)
        wv_sb = const.tile([128, 8, DC], BF16)
        wo_sb = const.tile([128, D], BF16)
        bq_sb = const.tile([128, 1], F32)
        bk_sb = const.tile([128, 1], F32)
        ones_sb = const.tile([128, 64], BF16)
        warm_sb = const.tile([128, 8], F32)
        nc.vector.memset(ones_sb[:], 1.0)
        nc.vector.memset(warm_sb[:], 0.0)
        nc.sync.dma_start(wq_sb[:], wq_d.rearrange("(c p) d -> p c d", p=128))
        nc.sync.dma_start(wk_sb[:], wk_d.rearrange("(c p) d -> p c d", p=128))
        nc.sync.dma_start(wv_sb[:], wv_d.rearrange("(c p) d -> p c d", p=128))
        nc.sync.dma_start(wo_sb[:], wo_d)
        nc.sync.dma_start(bq_sb[:], bq_d)
        nc.sync.dma_start(bk_sb[:], bk_d)
        nc.vector.memset(VE[:], 0.0)
        nc.vector.memset(VE[:, :, 0, 64:65], 1.0)
        nc.vector.memset(VE[:, :, 0, 68:69], 1.0)
        # preload the exp table set during the DMA-bound lead-in
        nc.scalar.activation(warm_sb[:], warm_sb[:], EXP, scale=1.0)

        # ---- work-unit emitters -----------------------------------------
        def emit_qk_unit(which, ss):
            """Project one 512-token slice of Q or K (8 accum matmuls)."""
            srcT, w_sb, b_sb, dstT = (
                (qT_d, wq_sb, bq_sb, QT) if which == "q"
                else (kT_d, wk_sb, bk_sb, KT))
            xt = stage.tile([128, 8, 512], BF16, tag="xT")
            nc.sync.dma_start(
                xt[:],
                srcT.rearrange("(c p) s -> p c s", p=128)[
                    :, :, ss * 512:(ss + 1) * 512],
            )
            ps = accp.tile([128, 512], F32, tag="acc", name="psqk")
            for c in range(8):
                nc.tensor.matmul(ps[:], lhsT=w_sb[:, c], rhs=xt[:, c],
                                 start=(c == 0), stop=(c == 7))
            nc.vector.tensor_scalar_add(
                dstT[:, ss * 512:(ss + 1) * 512], ps[:], b_sb[:])

        def emit_v_load(ss):
            """DMA one 512-token slice of v; returns the staged tile."""
            vt = stage.tile([128, 8, 512], BF16, tag="xT")
            nc.sync.dma_start(
                vt[:],
                vT_d.rearrange("(c p) s -> p c s", p=128)[
                    :, :, ss * 512:(ss + 1) * 512],
            )
            return vt

        def emit_v_unit(vt, ss, st):
            """Project one 128-token chunk of V into the VE layout."""
            chunk = ss * 4 + st
            ps = accp.tile([128, 512], F32, tag="acc", name="psv")
            for c in range(8):
                nc.tensor.matmul(
                    ps[:, 0:DC],
                    lhsT=vt[:, c, st * 128:(st + 1) * 128],
                    rhs=wv_sb[:, c],
                    start=(c == 0), stop=(c == 7))
            nc.vector.tensor_copy(
                VE[:, chunk, :, 0:64],
                ps[:, 0:DC].rearrange("p (a x) -> p a x", a=2))

        def emit_outproj_unit(st_abs):
            """Out-projection for one 128-token tile (2 matmuls, bf16 out)."""
            s0 = st_abs * 128
            osb = ostage.tile([128, D], BF16, tag="osb")
            for ns in range(2):
                ps3 = accp.tile([128, 512], F32, tag="acc", name="ps3")
                nc.tensor.matmul(
                    ps3[:],
                    lhsT=OT[:, s0:s0 + 128],
                    rhs=wo_sb[:, ns * 512:(ns + 1) * 512],
                    start=True, stop=True)
                nc.vector.tensor_copy(osb[:, ns * 512:(ns + 1) * 512], ps3[:])
            nc.sync.dma_start(out_d[s0:s0 + 128, :], osb[:])

        def emit_attention_qs(b, qs, fillers):
            """Attention for one (batch, 512-q-slice); pops one filler unit
            per chunk to keep the PE dense while ACT runs the exps."""
            q0 = b * S + qs * 512
            po = [pop.tile([128, 512], F32, tag="po", name=f"po{h}")
                  for h in range(HPC)]

            def emit_pv(chunk, pt):
                ve_flat = VE[:, b * 16 + chunk, :, :].rearrange(
                    "p a x -> p (a x)")
                first = chunk == 0
                last = chunk == 15
                # h0: rows 0:64 = O^T_h0, row 64 = rowsum_h0
                nc.tensor.matmul(
                    po[0][0:65, :], lhsT=ve_flat[:, 0:65], rhs=pt[:, 0],
                    start=first, stop=last)
                # h1: row 0 = rowsum_h1 (ones at abs 68), rows 64:128 = O^T_h1
                nc.tensor.matmul(
                    po[1][:, :], lhsT=ve_flat[:, 68:196], rhs=pt[:, 1],
                    start=first, stop=last)

            prev_pt = None
            for chunk in range(16):
                k0 = b * S + chunk * 128
                psc = pscp.tile([128, 2, 512], F32, tag="sc", name="psc")
                # row-tiled concurrent pair: h0 rows 0:64, h1 rows 64:128
                nc.tensor.matmul(
                    psc[:, 0], lhsT=KT[0:64, k0:k0 + 128],
                    rhs=QT[0:64, q0:q0 + 512], start=True, stop=True)
                nc.tensor.matmul(
                    psc[:, 1], lhsT=KT[64:128, k0:k0 + 128],
                    rhs=QT[64:128, q0:q0 + 512], start=True, stop=True)
                pt = ptpool.tile([128, 2, 512], BF16, tag="pt")
                nc.scalar.activation(
                    pt.rearrange("p a x -> p (a x)"),
                    psc.rearrange("p a x -> p (a x)"),
                    EXP, scale=0.125)
                if prev_pt is not None:
                    emit_pv(chunk - 1, prev_pt)
                if chunk >= 1:
                    run_unit(fillers)
                prev_pt = pt
            emit_pv(15, prev_pt)

            # normalization: fast reciprocal of the row-sum rows (bf16 cast so
            # the broadcast matmul's moving operand is bf16: fp32 rhs streams
            # at 4 cycles/row), col-tiled concurrent ones-broadcast, one
            # GPSIMD eviction copy, two DVE multiplies
            rr = npool.tile([128, 512], F32, tag="rr")
            nc.vector.reciprocal(rr[64:65, :], po[0][64:65, :])
            nc.vector.reciprocal(rr[0:1, :], po[1][0:1, :])
            rrb = npool.tile([128, 512], BF16, tag="rrb")
            nc.vector.tensor_copy(rrb[64:65, :], rr[64:65, :])
            nc.vector.tensor_copy(rrb[0:1, :], rr[0:1, :])
            bcp = accp.tile([128, 512], F32, tag="acc", name="bcp")
            nc.tensor.matmul(bcp[0:64, :], lhsT=ones_sb[64:65, :],
                             rhs=rrb[64:65, :], start=True, stop=True)
            nc.tensor.matmul(bcp[64:128, :], lhsT=ones_sb[0:1, :],
                             rhs=rrb[0:1, :], start=True, stop=True)
            bc = npool.tile([128, 512], F32, tag="bc")
            nc.vector.tensor_copy(bc[:], bcp[:])
            nc.vector.tensor_mul(
                OT[0:64, q0:q0 + 512], po[0][0:64, :], bc[0:64, :])
            nc.vector.tensor_mul(
                OT[64:128, q0:q0 + 512], po[1][64:128, :], bc[64:128, :])

        def run_unit(fillers):
            """Pop and emit one filler unit; a unit may return a list of
            follow-up units which are queued to run next (in order)."""
            if not fillers:
                return
            u = fillers.pop(0)
            r = u()
            if isinstance(r, list):
                fillers[0:0] = r

        def proj_units_for_batch(b):
            units = []
            for ss_local in range(4):
                ss = b * 4 + ss_local
                units.append(lambda ss=ss: emit_qk_unit("q", ss))
                units.append(lambda ss=ss: emit_qk_unit("k", ss))
            for ss_local in range(4):
                ss = b * 4 + ss_local

                def v_group(ss=ss):
                    vt = emit_v_load(ss)
                    return [lambda st=st, vt=vt, ss=ss: emit_v_unit(vt, ss, st)
                            for st in range(4)]
                units.append(v_group)
            return units

        # ---- main pipeline ----------------------------------------------
        # lead-in: project batch 0 eagerly (DMA-bound)
        lead = proj_units_for_batch(0)
        while lead:
            run_unit(lead)

        for b in range(B):
            fillers = []
            pending_proj = proj_units_for_batch(b + 1) if b + 1 < B else []
            for qs in range(4):
                # fillers: previous q-slice's out-projection tiles first
                # (their OT range was normalized at the end of the last qs)
                if qs > 0:
                    prev = (b * S + (qs - 1) * 512) // 128
                elif b > 0:
                    prev = ((b - 1) * S + 3 * 512) // 128
                else:
                    prev = None
                if prev is not None:
                    for k in range(4):
                        fillers.append(
                            lambda st=prev + k: emit_outproj_unit(st))
                # then a share of the next batch's projection units
                for _ in range(min(3, len(pending_proj))):
                    fillers.append(pending_proj.pop(0))

                emit_attention_qs(b, qs, fillers)

            # drain any unfinished fillers / projections at batch end
            while fillers:
                run_unit(fillers)
            while pending_proj:
                run_unit(pending_proj)

        # tail: out-projection of the last q-slice
        for k in range(4):
            emit_outproj_unit((3 * S + 3 * 512) // 128 + k)

    nc.compile()
    return nc


def _get_program():
    global _BUILT
    if _BUILT is None:
        _BUILT = _build_program()
    return _BUILT


def kernel(q, k, v, Wq, bq, Wk, bk, Wv, bv, Wo, bo, trace=None):
    global LAST_EXEC_NS, LAST_RESULTS
    if trace is None:
        trace = os.environ.get("KERNEL_TRACE", "0") == "1"
    bf16 = ml_dtypes.bfloat16

    q2 = np.asarray(q, np.float32).reshape(BS, D)
    k2 = np.asarray(k, np.float32).reshape(BS, D)
    v2 = np.asarray(v, np.float32).reshape(BS, D)
    qT = np.ascontiguousarray(q2.T).astype(bf16)
    kT = np.ascontiguousarray(k2.T).astype(bf16)
    vT = np.ascontiguousarray(v2.T).astype(bf16)

    Wq = np.asarray(Wq, np.float32)
    Wk = np.asarray(Wk, np.float32)
    Wv = np.asarray(Wv, np.float32)
    Wo = np.asarray(Wo, np.float32)
    bq = np.asarray(bq, np.float32)
    bk = np.asarray(bk, np.float32)
    bv = np.asarray(bv, np.float32)
    bo = np.asarray(bo, np.float32)

    in_maps = []
    for c in range(NCORES):
        sl = slice(c * DC, (c + 1) * DC)
        in_maps.append({
            "qT": qT, "kT": kT, "vT": vT,
            "wq": np.ascontiguousarray(Wq[:, sl]).astype(bf16),
            "wk": np.ascontiguousarray(Wk[:, sl]).astype(bf16),
            "wv": np.ascontiguousarray(Wv[:, sl]).astype(bf16),
            "wo": np.ascontiguousarray(Wo[sl, :]).astype(bf16),
            "bq": np.ascontiguousarray(bq[sl]).reshape(DC, 1),
            "bk": np.ascontiguousarray(bk[sl]).reshape(DC, 1),
        })

    nc = _get_program()
    res = run_bass_kernel_spmd(nc, in_maps, list(range(NCORES)), trace=trace)
    LAST_EXEC_NS = res.exec_time_ns
    LAST_RESULTS = res

    out = np.zeros((BS, D), np.float32)
    for c in range(NCORES):
        out += np.asarray(res.results[c]["out"], np.float32)
    out += bv.astype(np.float32) @ Wo + bo          # exact bias identities
    return out.reshape(B, S, D)


# revision 17
# speedup vs baseline: 1.3680x; 1.0911x over previous
"""Multi-head attention (B=4, S=2048, D=1024, H=16, Hd=64) on 8 NeuronCores.

Sharding: tensor-parallel over heads. Core c owns heads {2c, 2c+1}, i.e. a
128-column slice of Wq/Wk/Wv and the matching 128-row slice of Wo. Each core
computes a full-shape partial output (its heads' contribution through the out
projection); the host sums the 8 partials (f32) plus the exact bias identities
(softmax rows sum to 1 -> bv@Wo + bo added on host; bk cancels in softmax but
is still applied on-device for free).

v2 structure (vs v1): the whole kernel is a single software pipeline over
batches, engineered to keep the PE array continuously busy so the HAM clock
gate stays at 2.4 GHz (v1 ran most matmuls at the cold 1.2 GHz rate):

  * per-batch projection -> attention -> out-projection, with projection and
    out-projection matmuls of neighbouring batches interleaved as "filler"
    units inside the attention chunk loop (PE never idles while ACT does exp).
  * scores for the two heads are issued as a row-tiled pair (h0 rows 0:64,
    h1 rows 64:128 via tile_position auto-derivation) into separate PSUM banks
    of one [128, 2, 512] tile -> they execute concurrently on the PE, and one
    ACT exp (N=1024) covers both heads.
  * softmax normalization: DVE reciprocal_approx_fast (0.7us vs 3.3us for the
    iterative reciprocal), then a col-tiled concurrent pair of K=1 ones-matmul
    broadcasts, one PSUM->SBUF copy, two DVE multiplies.
  * out-projection eviction entirely on DVE (v1 put half on ACT, the exp
    engine), output written bf16 (halves write traffic; host sums in f32).

Device algorithm per core (all matmuls bf16, f32 PSUM):
  1. QT/KT = Wc^T x^T + b  -> SBUF [128=d', 8192=s] bf16 (h0 rows 0:64,
     h1 rows 64:128); V -> SBUF [token, d'] chunks with ones columns for the
     softmax row-sum rows (VE layout [128, chunk, 2, 132]).
  2. Per (batch, q-slice of 512): 16 k-chunks of 128: scores^T pair ->
     exp -> P^T; O^T accumulated per head via [V_h | ones] lhsT (row-sum row
     rides along). Normalize with reciprocal + PE broadcast -> OT bf16.
  3. out_partial = OT^T @ Wo per s-tile -> DRAM bf16.
"""

import os
from contextlib import ExitStack

import numpy as np
import ml_dtypes

import concourse.bass as bass
import concourse.mybir as mybir
import concourse.tile as tile
from concourse import bacc, library_config
from concourse.bass_utils import run_bass_kernel_spmd

B, S, D, H, HD = 4, 2048, 1024, 16, 64
BS = B * S                     # 8192 flattened tokens
NCORES = 8
HPC = H // NCORES              # 2 heads per core
DC = HPC * HD                  # 128-wide weight slice per core

F32 = mybir.dt.float32
BF16 = mybir.dt.bfloat16
EXP = mybir.ActivationFunctionType.Exp
LOG = mybir.ActivationFunctionType.Ln

_BUILT = None
LAST_EXEC_NS = None
LAST_RESULTS = None


def _build_program():
    nc = bacc.Bacc("TRN2", target_bir_lowering=False, debug=False,
                   num_devices=NCORES)

    qT_d = nc.dram_tensor("qT", [D, BS], BF16, kind="ExternalInput").ap()
    kT_d = nc.dram_tensor("kT", [D, BS], BF16, kind="ExternalInput").ap()
    vT_d = nc.dram_tensor("vT", [D, BS], BF16, kind="ExternalInput").ap()
    wq_d = nc.dram_tensor("wq", [D, DC], BF16, kind="ExternalInput").ap()
    wk_d = nc.dram_tensor("wk", [D, DC], BF16, kind="ExternalInput").ap()
    wv_d = nc.dram_tensor("wv", [D, DC], BF16, kind="ExternalInput").ap()
    wo_d = nc.dram_tensor("wo", [DC, D], BF16, kind="ExternalInput").ap()
    bq_d = nc.dram_tensor("bq", [DC, 1], F32, kind="ExternalInput").ap()
    bk_d = nc.dram_tensor("bk", [DC, 1], F32, kind="ExternalInput").ap()
    out_d = nc.dram_tensor("out", [BS, D], BF16, kind="ExternalOutput").ap()

    with tile.TileContext(nc) as tc, ExitStack() as ctx:
        const = ctx.enter_context(tc.tile_pool(name="const", bufs=1))
        persist = ctx.enter_context(tc.tile_pool(name="persist", bufs=1))
        stage = ctx.enter_context(tc.tile_pool(name="stage", bufs=3))
        ptpool = ctx.enter_context(tc.tile_pool(name="ptpool", bufs=4))
        npool = ctx.enter_context(tc.tile_pool(name="npool", bufs=4))
        ostage = ctx.enter_context(tc.tile_pool(name="ostage", bufs=3))
        # PSUM: psc 2 slots x 2 banks (scores double-buffer)
        #       acc 2 slots x 1 bank (proj accum / out-proj / bcast)
        #       pop 2 slots x 1 bank (per-head O^T accumulators)
        pscp = ctx.enter_context(tc.tile_pool(name="pscp", bufs=2, space="PSUM"))
        accp = ctx.enter_context(tc.tile_pool(name="accp", bufs=2, space="PSUM"))
        pop = ctx.enter_context(tc.tile_pool(name="pop", bufs=2, space="PSUM"))

        # ---- persistent SBUF state -------------------------------------
        QT = persist.tile([128, BS], BF16)          # [d', s]
        KT = persist.tile([128, BS], BF16)
        OT = persist.tile([128, BS], BF16)
        # V extended, per 128-token chunk (free layout [2, 132], abs width 264):
        #   abs cols 0:64    = V_h0          (h0 lhsT = abs 0:65, rsum row 64)
        #   abs col  64      = ones
        #   abs col  68      = ones          (h1 lhsT = abs 68:196, rsum row 0)
        #   abs cols 132:196 = V_h1          (-> h1 lhsT rows 64:128)
        VE = persist.tile([128, 64, 2, 132], BF16)

        # ---- constants --------------------------------------------------
        wq_sb = const.tile([128, 8, DC], BF16)
        wk_sb = const.tile([128, 8, DC], BF16)
        wv_sb = const.tile([128, 8, DC], BF16)
        wo_sb = const.tile([128, D], BF16)
        bq_sb = const.tile([128, 1], F32)
        bk_sb = const.tile([128, 1], F32)
        ones_sb = const.tile([128, 64], BF16)
        warm_sb = const.tile([128, 8], F32)
        nc.vector.memset(ones_sb[:], 1.0)
        nc.vector.memset(warm_sb[:], 0.0)
        nc.sync.dma_start(wq_sb[:], wq_d.rearrange("(c p) d -> p c d", p=128))
        nc.sync.dma_start(wk_sb[:], wk_d.rearrange("(c p) d -> p c d", p=128))
        nc.sync.dma_start(wv_sb[:], wv_d.rearrange("(c p) d -> p c d", p=128))
        nc.sync.dma_start(wo_sb[:], wo_d)
        nc.sync.dma_start(bq_sb[:], bq_d)
        nc.sync.dma_start(bk_sb[:], bk_d)
        nc.vector.memset(VE[:], 0.0)
        nc.vector.memset(VE[:, :, 0, 64:65], 1.0)
        nc.vector.memset(VE[:, :, 0, 68:69], 1.0)
        # preload the exp+log table set during the DMA-bound lead-in
        nc.scalar.activation(warm_sb[:], warm_sb[:], EXP, scale=1.0)
        nc.scalar.activation(warm_sb[:], warm_sb[:], LOG, scale=1.0)

        # ---- work-unit emitters -----------------------------------------
        def emit_qk_unit(which, ss):
            """Project one 512-token slice of Q or K (8 accum matmuls)."""
            srcT, w_sb, b_sb, dstT = (
                (qT_d, wq_sb, bq_sb, QT) if which == "q"
                else (kT_d, wk_sb, bk_sb, KT))
            xt = stage.tile([128, 8, 512], BF16, tag="xT")
            nc.sync.dma_start(
                xt[:],
                srcT.rearrange("(c p) s -> p c s", p=128)[
                    :, :, ss * 512:(ss + 1) * 512],
            )
            ps = accp.tile([128, 512], F32, tag="acc", name="psqk")
            for c in range(8):
                nc.tensor.matmul(ps[:], lhsT=w_sb[:, c], rhs=xt[:, c],
                                 start=(c == 0), stop=(c == 7))
            nc.vector.tensor_scalar_add(
                dstT[:, ss * 512:(ss + 1) * 512], ps[:], b_sb[:])

        def emit_v_load(ss):
            """DMA one 512-token slice of v; returns the staged tile."""
            vt = stage.tile([128, 8, 512], BF16, tag="xT")
            nc.sync.dma_start(
                vt[:],
                vT_d.rearrange("(c p) s -> p c s", p=128)[
                    :, :, ss * 512:(ss + 1) * 512],
            )
            return vt

        def emit_v_unit(vt, ss, st):
            """Project one 128-token chunk of V into the VE layout."""
            chunk = ss * 4 + st
            ps = accp.tile([128, 512], F32, tag="acc", name="psv")
            for c in range(8):
                nc.tensor.matmul(
                    ps[:, 0:DC],
                    lhsT=vt[:, c, st * 128:(st + 1) * 128],
                    rhs=wv_sb[:, c],
                    start=(c == 0), stop=(c == 7))
            nc.vector.tensor_copy(
                VE[:, chunk, :, 0:64],
                ps[:, 0:DC].rearrange("p (a x) -> p a x", a=2))

        def emit_outproj_unit(st_abs):
            """Out-projection for one 128-token tile (2 matmuls, bf16 out)."""
            s0 = st_abs * 128
            osb = ostage.tile([128, D], BF16, tag="osb")
            for ns in range(2):
                ps3 = accp.tile([128, 512], F32, tag="acc", name="ps3")
                nc.tensor.matmul(
                    ps3[:],
                    lhsT=OT[:, s0:s0 + 128],
                    rhs=wo_sb[:, ns * 512:(ns + 1) * 512],
                    start=True, stop=True)
                if ns == 0:
                    nc.vector.tensor_copy(osb[:, 0:512], ps3[:])
                else:
                    nc.scalar.copy(osb[:, 512:1024], ps3[:])
            nc.sync.dma_start(out_d[s0:s0 + 128, :], osb[:])

        def emit_attention_qs(b, qs, fillers):
            """Attention for one (batch, 512-q-slice); pops one filler unit
            per chunk to keep the PE dense while ACT runs the exps."""
            q0 = b * S + qs * 512
            po = [pop.tile([128, 512], F32, tag="po", name=f"po{h}")
                  for h in range(HPC)]

            def emit_pv(chunk, pt):
                ve_flat = VE[:, b * 16 + chunk, :, :].rearrange(
                    "p a x -> p (a x)")
                first = chunk == 0
                last = chunk == 15
                # h0: rows 0:64 = O^T_h0, row 64 = rowsum_h0
                nc.tensor.matmul(
                    po[0][0:65, :], lhsT=ve_flat[:, 0:65], rhs=pt[:, 0],
                    start=first, stop=last)
                # h1: row 0 = rowsum_h1 (ones at abs 68), rows 64:128 = O^T_h1
                nc.tensor.matmul(
                    po[1][:, :], lhsT=ve_flat[:, 68:196], rhs=pt[:, 1],
                    start=first, stop=last)

            prev_pt = None
            for chunk in range(16):
                k0 = b * S + chunk * 128
                psc = pscp.tile([128, 2, 512], F32, tag="sc", name="psc")
                # row-tiled concurrent pair: h0 rows 0:64, h1 rows 64:128
                nc.tensor.matmul(
                    psc[:, 0], lhsT=KT[0:64, k0:k0 + 128],
                    rhs=QT[0:64, q0:q0 + 512], start=True, stop=True)
                nc.tensor.matmul(
                    psc[:, 1], lhsT=KT[64:128, k0:k0 + 128],
                    rhs=QT[64:128, q0:q0 + 512], start=True, stop=True)
                pt = ptpool.tile([128, 2, 512], BF16, tag="pt")
                nc.scalar.activation(
                    pt.rearrange("p a x -> p (a x)"),
                    psc.rearrange("p a x -> p (a x)"),
                    EXP, scale=0.125)
                if prev_pt is not None:
                    emit_pv(chunk - 1, prev_pt)
                if chunk >= 1:
                    run_unit(fillers)
                prev_pt = pt
            emit_pv(15, prev_pt)

            # normalization: fast reciprocal of the row-sum rows (bf16 cast so
            # the broadcast matmul's moving operand is bf16: fp32 rhs streams
            # at 4 cycles/row), col-tiled concurrent ones-broadcast, one
            # GPSIMD eviction copy, two DVE multiplies
            # 1/rowsum = exp(-ln rowsum) on ACT: Log and Exp live in one
            # table set (natural_log_exp_and_others) so there is no table
            # switching, and this keeps the slow iterative reciprocal off
            # the DVE (3.35us per row there)
            rl = npool.tile([128, 512], F32, tag="rl")
            nc.scalar.activation(rl[64:65, :], po[0][64:65, :], LOG)
            nc.scalar.activation(rl[0:1, :], po[1][0:1, :], LOG)
            rrb = npool.tile([128, 512], BF16, tag="rrb")
            nc.scalar.activation(rrb[64:65, :], rl[64:65, :], EXP, scale=-1.0)
            nc.scalar.activation(rrb[0:1, :], rl[0:1, :], EXP, scale=-1.0)
            bcp = accp.tile([128, 512], F32, tag="acc", name="bcp")
            nc.tensor.matmul(bcp[0:64, :], lhsT=ones_sb[64:65, :],
                             rhs=rrb[64:65, :], start=True, stop=True)
            nc.tensor.matmul(bcp[64:128, :], lhsT=ones_sb[0:1, :],
                             rhs=rrb[0:1, :], start=True, stop=True)
            bc = npool.tile([128, 512], F32, tag="bc")
            nc.vector.tensor_copy(bc[:], bcp[:])
            nc.vector.tensor_mul(
                OT[0:64, q0:q0 + 512], po[0][0:64, :], bc[0:64, :])
            nc.vector.tensor_mul(
                OT[64:128, q0:q0 + 512], po[1][64:128, :], bc[64:128, :])

        def run_unit(fillers):
            """Pop and emit one filler unit; a unit may return a list of
            follow-up units which are queued to run next (in order)."""
            if not fillers:
                return
            u = fillers.pop(0)
            r = u()
            if isinstance(r, list):
                fillers[0:0] = r

        def proj_units_for_batch(b):
            # interleave [Q, K, V] per token-slice so the first attention
            # chunks of the batch unblock after ~1/4 of the batch's DMA, and
            # V work spreads evenly across the previous batch's q-slices
            units = []
            for ss_local in range(4):
                ss = b * 4 + ss_local
                units.append(lambda ss=ss: emit_qk_unit("k", ss))
                units.append(lambda ss=ss: emit_qk_unit("q", ss))

                def v_group(ss=ss):
                    vt = emit_v_load(ss)
                    return [lambda st=st, vt=vt, ss=ss: emit_v_unit(vt, ss, st)
                            for st in range(4)]
                units.append(v_group)
            return units

        # ---- main pipeline ----------------------------------------------
        # lead-in: project batch 0 eagerly (DMA-bound)
        lead = proj_units_for_batch(0)
        while lead:
            run_unit(lead)

        for b in range(B):
            fillers = []
            pending_proj = proj_units_for_batch(b + 1) if b + 1 < B else []
            for qs in range(4):
                # fillers: previous q-slice's out-projection tiles first
                # (their OT range was normalized at the end of the last qs)
                if qs > 0:
                    prev = (b * S + (qs - 1) * 512) // 128
                elif b > 0:
                    prev = ((b - 1) * S + 3 * 512) // 128
                else:
                    prev = None
                if prev is not None:
                    for k in range(4):
                        fillers.append(
                            lambda st=prev + k: emit_outproj_unit(st))
                # then a share of the next batch's projection units
                for _ in range(min(3, len(pending_proj))):
                    fillers.append(pending_proj.pop(0))

                emit_attention_qs(b, qs, fillers)

            # drain any unfinished fillers / projections at batch end
            while fillers:
                run_unit(fillers)
            while pending_proj:
                run_unit(pending_proj)

        # tail: out-projection of the last q-slice
        for k in range(4):
            emit_outproj_unit((3 * S + 3 * 512) // 128 + k)

    nc.compile()
    return nc


def _get_program():
    global _BUILT
    if _BUILT is None:
        _BUILT = _build_program()
    return _BUILT


def kernel(q, k, v, Wq, bq, Wk, bk, Wv, bv, Wo, bo, trace=None):
    global LAST_EXEC_NS, LAST_RESULTS
    if trace is None:
        trace = os.environ.get("KERNEL_TRACE", "0") == "1"
    bf16 = ml_dtypes.bfloat16

    q2 = np.asarray(q, np.float32).reshape(BS, D)
    k2 = np.asarray(k, np.float32).reshape(BS, D)
    v2 = np.asarray(v, np.float32).reshape(BS, D)
    qT = np.ascontiguousarray(q2.T).astype(bf16)
    kT = np.ascontiguousarray(k2.T).astype(bf16)
    vT = np.ascontiguousarray(v2.T).astype(bf16)

    Wq = np.asarray(Wq, np.float32)
    Wk = np.asarray(Wk, np.float32)
    Wv = np.asarray(Wv, np.float32)
    Wo = np.asarray(Wo, np.float32)
    bq = np.asarray(bq, np.float32)
    bk = np.asarray(bk, np.float32)
    bv = np.asarray(bv, np.float32)
    bo = np.asarray(bo, np.float32)

    in_maps = []
    for c in range(NCORES):
        sl = slice(c * DC, (c + 1) * DC)
        in_maps.append({
            "qT": qT, "kT": kT, "vT": vT,
            "wq": np.ascontiguousarray(Wq[:, sl]).astype(bf16),
            "wk": np.ascontiguousarray(Wk[:, sl]).astype(bf16),
            "wv": np.ascontiguousarray(Wv[:, sl]).astype(bf16),
            "wo": np.ascontiguousarray(Wo[sl, :]).astype(bf16),
            "bq": np.ascontiguousarray(bq[sl]).reshape(DC, 1),
            "bk": np.ascontiguousarray(bk[sl]).reshape(DC, 1),
        })

    nc = _get_program()
    res = run_bass_kernel_spmd(nc, in_maps, list(range(NCORES)), trace=trace)
    LAST_EXEC_NS = res.exec_time_ns
    LAST_RESULTS = res

    out = np.zeros((BS, D), np.float32)
    for c in range(NCORES):
        out += np.asarray(res.results[c]["out"], np.float32)
    out += bv.astype(np.float32) @ Wo + bo          # exact bias identities
    return out.reshape(B, S, D)


# revision 18
# speedup vs baseline: 1.4783x; 1.0806x over previous
"""Multi-head attention (B=4, S=2048, D=1024, H=16, Hd=64) on 8 NeuronCores.

Sharding: tensor-parallel over heads. Core c owns heads {2c, 2c+1}, i.e. a
128-column slice of Wq/Wk/Wv and the matching 128-row slice of Wo. Each core
computes a full-shape partial output (its heads' contribution through the out
projection); the host sums the 8 partials (f32) plus the exact bias identities
(softmax rows sum to 1 -> bv@Wo + bo added on host; bk cancels in softmax but
is still applied on-device for free).

v2 structure (vs v1): the whole kernel is a single software pipeline over
batches, engineered to keep the PE array continuously busy so the HAM clock
gate stays at 2.4 GHz (v1 ran most matmuls at the cold 1.2 GHz rate):

  * per-batch projection -> attention -> out-projection, with projection and
    out-projection matmuls of neighbouring batches interleaved as "filler"
    units inside the attention chunk loop (PE never idles while ACT does exp).
  * scores for the two heads are issued as a row-tiled pair (h0 rows 0:64,
    h1 rows 64:128 via tile_position auto-derivation) into separate PSUM banks
    of one [128, 2, 512] tile -> they execute concurrently on the PE, and one
    ACT exp (N=1024) covers both heads.
  * softmax normalization: DVE reciprocal_approx_fast (0.7us vs 3.3us for the
    iterative reciprocal), then a col-tiled concurrent pair of K=1 ones-matmul
    broadcasts, one PSUM->SBUF copy, two DVE multiplies.
  * out-projection eviction entirely on DVE (v1 put half on ACT, the exp
    engine), output written bf16 (halves write traffic; host sums in f32).

Device algorithm per core (all matmuls bf16, f32 PSUM):
  1. QT/KT = Wc^T x^T + b  -> SBUF [128=d', 8192=s] bf16 (h0 rows 0:64,
     h1 rows 64:128); V -> SBUF [token, d'] chunks with ones columns for the
     softmax row-sum rows (VE layout [128, chunk, 2, 132]).
  2. Per (batch, q-slice of 512): 16 k-chunks of 128: scores^T pair ->
     exp -> P^T; O^T accumulated per head via [V_h | ones] lhsT (row-sum row
     rides along). Normalize with reciprocal + PE broadcast -> OT bf16.
  3. out_partial = OT^T @ Wo per s-tile -> DRAM bf16.
"""

import os
from contextlib import ExitStack

import numpy as np
import ml_dtypes

import concourse.bass as bass
import concourse.mybir as mybir
import concourse.tile as tile
from concourse import bacc, library_config
from concourse.bass_utils import run_bass_kernel_spmd

B, S, D, H, HD = 4, 2048, 1024, 16, 64
BS = B * S                     # 8192 flattened tokens
NCORES = 8
HPC = H // NCORES              # 2 heads per core
DC = HPC * HD                  # 128-wide weight slice per core

F32 = mybir.dt.float32
BF16 = mybir.dt.bfloat16
EXP = mybir.ActivationFunctionType.Exp
LOG = mybir.ActivationFunctionType.Ln

_BUILT = None
LAST_EXEC_NS = None
LAST_RESULTS = None


def _patch_act_tables():
    """Steer the ACT table-load inserter to the combined exp+ln set.

    The inserter greedily picks the first act-func set containing each
    activation's function, so Exp lands in `exp_and_others` and Ln in
    `natural_log` and the kernel thrashes 2.7us table loads every q-slice.
    Presenting Exp/Ln as available only in `natural_log_exp_and_others`
    (which genuinely contains both) yields a single load at kernel start.
    """
    import concourse.bacc as bacc_mod

    orig = bacc_mod.get_activation_tables
    if getattr(orig, "_combined_exp_ln", False):
        return
    def patched(arch):
        tables = orig(arch)
        for name, fns in tables.items():
            if name != "natural_log_exp_and_others":
                fns.discard(mybir.ActivationFunctionType.Exp)
                fns.discard(mybir.ActivationFunctionType.Ln)
        return tables
    patched._combined_exp_ln = True
    bacc_mod.get_activation_tables = patched


def _build_program():
    _patch_act_tables()
    nc = bacc.Bacc("TRN2", target_bir_lowering=False, debug=False,
                   num_devices=NCORES)

    qT_d = nc.dram_tensor("qT", [D, BS], BF16, kind="ExternalInput").ap()
    kT_d = nc.dram_tensor("kT", [D, BS], BF16, kind="ExternalInput").ap()
    vT_d = nc.dram_tensor("vT", [D, BS], BF16, kind="ExternalInput").ap()
    wq_d = nc.dram_tensor("wq", [D, DC], BF16, kind="ExternalInput").ap()
    wk_d = nc.dram_tensor("wk", [D, DC], BF16, kind="ExternalInput").ap()
    wv_d = nc.dram_tensor("wv", [D, DC], BF16, kind="ExternalInput").ap()
    wo_d = nc.dram_tensor("wo", [DC, D], BF16, kind="ExternalInput").ap()
    bq_d = nc.dram_tensor("bq", [DC, 1], F32, kind="ExternalInput").ap()
    bk_d = nc.dram_tensor("bk", [DC, 1], F32, kind="ExternalInput").ap()
    out_d = nc.dram_tensor("out", [BS, D], BF16, kind="ExternalOutput").ap()

    with tile.TileContext(nc) as tc, ExitStack() as ctx:
        const = ctx.enter_context(tc.tile_pool(name="const", bufs=1))
        persist = ctx.enter_context(tc.tile_pool(name="persist", bufs=1))
        stage = ctx.enter_context(tc.tile_pool(name="stage", bufs=3))
        ptpool = ctx.enter_context(tc.tile_pool(name="ptpool", bufs=4))
        npool = ctx.enter_context(tc.tile_pool(name="npool", bufs=4))
        ostage = ctx.enter_context(tc.tile_pool(name="ostage", bufs=3))
        # PSUM: psc 2 slots x 2 banks (scores double-buffer)
        #       acc 2 slots x 1 bank (proj accum / out-proj / bcast)
        #       pop 2 slots x 1 bank (per-head O^T accumulators)
        pscp = ctx.enter_context(tc.tile_pool(name="pscp", bufs=2, space="PSUM"))
        accp = ctx.enter_context(tc.tile_pool(name="accp", bufs=2, space="PSUM"))
        pop = ctx.enter_context(tc.tile_pool(name="pop", bufs=2, space="PSUM"))

        # ---- persistent SBUF state -------------------------------------
        QT = persist.tile([128, BS], BF16)          # [d', s]
        KT = persist.tile([128, BS], BF16)
        OT = persist.tile([128, BS], BF16)
        # V extended, per 128-token chunk (free layout [2, 132], abs width 264):
        #   abs cols 0:64    = V_h0          (h0 lhsT = abs 0:65, rsum row 64)
        #   abs col  64      = ones
        #   abs col  68      = ones          (h1 lhsT = abs 68:196, rsum row 0)
        #   abs cols 132:196 = V_h1          (-> h1 lhsT rows 64:128)
        VE = persist.tile([128, 64, 2, 132], BF16)

        # ---- constants --------------------------------------------------
        wq_sb = const.tile([128, 8, DC], BF16)
        wk_sb = const.tile([128, 8, DC], BF16)
        wv_sb = const.tile([128, 8, DC], BF16)
        wo_sb = const.tile([128, D], BF16)
        bq_sb = const.tile([128, 1], F32)
        bk_sb = const.tile([128, 1], F32)
        ones_sb = const.tile([128, 64], BF16)
        warm_sb = const.tile([128, 8], F32)
        nc.vector.memset(ones_sb[:], 1.0)
        nc.vector.memset(warm_sb[:], 0.0)
        nc.sync.dma_start(wq_sb[:], wq_d.rearrange("(c p) d -> p c d", p=128))
        nc.sync.dma_start(wk_sb[:], wk_d.rearrange("(c p) d -> p c d", p=128))
        nc.sync.dma_start(wv_sb[:], wv_d.rearrange("(c p) d -> p c d", p=128))
        nc.sync.dma_start(wo_sb[:], wo_d)
        nc.sync.dma_start(bq_sb[:], bq_d)
        nc.sync.dma_start(bk_sb[:], bk_d)
        nc.vector.memset(VE[:], 0.0)
        nc.vector.memset(VE[:, :, 0, 64:65], 1.0)
        nc.vector.memset(VE[:, :, 0, 68:69], 1.0)
        # preload the exp+log table set during the DMA-bound lead-in
        nc.scalar.activation(warm_sb[:], warm_sb[:], EXP, scale=1.0)
        nc.scalar.activation(warm_sb[:], warm_sb[:], LOG, scale=1.0)

        # ---- work-unit emitters -----------------------------------------
        def emit_qk_unit(which, ss):
            """Project one 512-token slice of Q or K (8 accum matmuls)."""
            srcT, w_sb, b_sb, dstT = (
                (qT_d, wq_sb, bq_sb, QT) if which == "q"
                else (kT_d, wk_sb, bk_sb, KT))
            xt = stage.tile([128, 8, 512], BF16, tag="xT")
            nc.sync.dma_start(
                xt[:],
                srcT.rearrange("(c p) s -> p c s", p=128)[
                    :, :, ss * 512:(ss + 1) * 512],
            )
            ps = accp.tile([128, 512], F32, tag="acc", name="psqk")
            for c in range(8):
                nc.tensor.matmul(ps[:], lhsT=w_sb[:, c], rhs=xt[:, c],
                                 start=(c == 0), stop=(c == 7))
            nc.vector.tensor_scalar_add(
                dstT[:, ss * 512:(ss + 1) * 512], ps[:], b_sb[:])

        def emit_v_load(ss):
            """DMA one 512-token slice of v; returns the staged tile."""
            vt = stage.tile([128, 8, 512], BF16, tag="xT")
            nc.sync.dma_start(
                vt[:],
                vT_d.rearrange("(c p) s -> p c s", p=128)[
                    :, :, ss * 512:(ss + 1) * 512],
            )
            return vt

        def emit_v_unit(vt, ss, st):
            """Project one 128-token chunk of V into the VE layout."""
            chunk = ss * 4 + st
            ps = accp.tile([128, 512], F32, tag="acc", name="psv")
            for c in range(8):
                nc.tensor.matmul(
                    ps[:, 0:DC],
                    lhsT=vt[:, c, st * 128:(st + 1) * 128],
                    rhs=wv_sb[:, c],
                    start=(c == 0), stop=(c == 7))
            nc.vector.tensor_copy(
                VE[:, chunk, :, 0:64],
                ps[:, 0:DC].rearrange("p (a x) -> p a x", a=2))

        def emit_outproj_unit(st_abs):
            """Out-projection for one 128-token tile (2 matmuls, bf16 out)."""
            s0 = st_abs * 128
            osb = ostage.tile([128, D], BF16, tag="osb")
            for ns in range(2):
                ps3 = accp.tile([128, 512], F32, tag="acc", name="ps3")
                nc.tensor.matmul(
                    ps3[:],
                    lhsT=OT[:, s0:s0 + 128],
                    rhs=wo_sb[:, ns * 512:(ns + 1) * 512],
                    start=True, stop=True)
                if ns == 0:
                    nc.vector.tensor_copy(osb[:, 0:512], ps3[:])
                else:
                    nc.scalar.copy(osb[:, 512:1024], ps3[:])
            nc.sync.dma_start(out_d[s0:s0 + 128, :], osb[:])

        def emit_attention_qs(b, qs, fillers):
            """Attention for one (batch, 512-q-slice); pops one filler unit
            per chunk to keep the PE dense while ACT runs the exps."""
            q0 = b * S + qs * 512
            po = [pop.tile([128, 512], F32, tag="po", name=f"po{h}")
                  for h in range(HPC)]

            def emit_pv(chunk, pt):
                ve_flat = VE[:, b * 16 + chunk, :, :].rearrange(
                    "p a x -> p (a x)")
                first = chunk == 0
                last = chunk == 15
                # h0: rows 0:64 = O^T_h0, row 64 = rowsum_h0
                nc.tensor.matmul(
                    po[0][0:65, :], lhsT=ve_flat[:, 0:65], rhs=pt[:, 0],
                    start=first, stop=last)
                # h1: row 0 = rowsum_h1 (ones at abs 68), rows 64:128 = O^T_h1
                nc.tensor.matmul(
                    po[1][:, :], lhsT=ve_flat[:, 68:196], rhs=pt[:, 1],
                    start=first, stop=last)

            prev_pt = None
            for chunk in range(16):
                k0 = b * S + chunk * 128
                psc = pscp.tile([128, 2, 512], F32, tag="sc", name="psc")
                # row-tiled concurrent pair: h0 rows 0:64, h1 rows 64:128
                nc.tensor.matmul(
                    psc[:, 0], lhsT=KT[0:64, k0:k0 + 128],
                    rhs=QT[0:64, q0:q0 + 512], start=True, stop=True)
                nc.tensor.matmul(
                    psc[:, 1], lhsT=KT[64:128, k0:k0 + 128],
                    rhs=QT[64:128, q0:q0 + 512], start=True, stop=True)
                pt = ptpool.tile([128, 2, 512], BF16, tag="pt")
                nc.scalar.activation(
                    pt.rearrange("p a x -> p (a x)"),
                    psc.rearrange("p a x -> p (a x)"),
                    EXP, scale=0.125)
                if prev_pt is not None:
                    emit_pv(chunk - 1, prev_pt)
                if chunk >= 1:
                    run_unit(fillers)
                prev_pt = pt
            emit_pv(15, prev_pt)

            # normalization: fast reciprocal of the row-sum rows (bf16 cast so
            # the broadcast matmul's moving operand is bf16: fp32 rhs streams
            # at 4 cycles/row), col-tiled concurrent ones-broadcast, one
            # GPSIMD eviction copy, two DVE multiplies
            # 1/rowsum = exp(-ln rowsum) on ACT: Log and Exp live in one
            # table set (natural_log_exp_and_others) so there is no table
            # switching, and this keeps the slow iterative reciprocal off
            # the DVE (3.35us per row there)
            rl = npool.tile([128, 512], F32, tag="rl")
            nc.scalar.activation(rl[64:65, :], po[0][64:65, :], LOG)
            nc.scalar.activation(rl[0:1, :], po[1][0:1, :], LOG)
            rrb = npool.tile([128, 512], BF16, tag="rrb")
            nc.scalar.activation(rrb[64:65, :], rl[64:65, :], EXP, scale=-1.0)
            nc.scalar.activation(rrb[0:1, :], rl[0:1, :], EXP, scale=-1.0)
            bcp = accp.tile([128, 512], F32, tag="acc", name="bcp")
            nc.tensor.matmul(bcp[0:64, :], lhsT=ones_sb[64:65, :],
                             rhs=rrb[64:65, :], start=True, stop=True)
            nc.tensor.matmul(bcp[64:128, :], lhsT=ones_sb[0:1, :],
                             rhs=rrb[0:1, :], start=True, stop=True)
            bc = npool.tile([128, 512], F32, tag="bc")
            nc.vector.tensor_copy(bc[:], bcp[:])
            nc.vector.tensor_mul(
                OT[0:64, q0:q0 + 512], po[0][0:64, :], bc[0:64, :])
            nc.vector.tensor_mul(
                OT[64:128, q0:q0 + 512], po[1][64:128, :], bc[64:128, :])

        def run_unit(fillers):
            """Pop and emit one filler unit; a unit may return a list of
            follow-up units which are queued to run next (in order)."""
            if not fillers:
                return
            u = fillers.pop(0)
            r = u()
            if isinstance(r, list):
                fillers[0:0] = r

        def proj_units_for_batch(b):
            # interleave [Q, K, V] per token-slice so the first attention
            # chunks of the batch unblock after ~1/4 of the batch's DMA, and
            # V work spreads evenly across the previous batch's q-slices
            units = []
            for ss_local in range(4):
                ss = b * 4 + ss_local
                units.append(lambda ss=ss: emit_qk_unit("k", ss))
                units.append(lambda ss=ss: emit_qk_unit("q", ss))

                def v_group(ss=ss):
                    vt = emit_v_load(ss)
                    return [lambda st=st, vt=vt, ss=ss: emit_v_unit(vt, ss, st)
                            for st in range(4)]
                units.append(v_group)
            return units

        # ---- main pipeline ----------------------------------------------
        # lead-in: project batch 0 eagerly (DMA-bound)
        lead = proj_units_for_batch(0)
        while lead:
            run_unit(lead)

        for b in range(B):
            fillers = []
            pending_proj = proj_units_for_batch(b + 1) if b + 1 < B else []
            for qs in range(4):
                # fillers: previous q-slice's out-projection tiles first
                # (their OT range was normalized at the end of the last qs)
                if qs > 0:
                    prev = (b * S + (qs - 1) * 512) // 128
                elif b > 0:
                    prev = ((b - 1) * S + 3 * 512) // 128
                else:
                    prev = None
                if prev is not None:
                    for k in range(4):
                        fillers.append(
                            lambda st=prev + k: emit_outproj_unit(st))
                # then a share of the next batch's projection units
                for _ in range(min(3, len(pending_proj))):
                    fillers.append(pending_proj.pop(0))

                emit_attention_qs(b, qs, fillers)

            # drain any unfinished fillers / projections at batch end
            while fillers:
                run_unit(fillers)
            while pending_proj:
                run_unit(pending_proj)

        # tail: out-projection of the last q-slice
        for k in range(4):
            emit_outproj_unit((3 * S + 3 * 512) // 128 + k)

    nc.compile()
    return nc


def _get_program():
    global _BUILT
    if _BUILT is None:
        _BUILT = _build_program()
    return _BUILT


def kernel(q, k, v, Wq, bq, Wk, bk, Wv, bv, Wo, bo, trace=None):
    global LAST_EXEC_NS, LAST_RESULTS
    if trace is None:
        trace = os.environ.get("KERNEL_TRACE", "0") == "1"
    bf16 = ml_dtypes.bfloat16

    q2 = np.asarray(q, np.float32).reshape(BS, D)
    k2 = np.asarray(k, np.float32).reshape(BS, D)
    v2 = np.asarray(v, np.float32).reshape(BS, D)
    qT = np.ascontiguousarray(q2.T).astype(bf16)
    kT = np.ascontiguousarray(k2.T).astype(bf16)
    vT = np.ascontiguousarray(v2.T).astype(bf16)

    Wq = np.asarray(Wq, np.float32)
    Wk = np.asarray(Wk, np.float32)
    Wv = np.asarray(Wv, np.float32)
    Wo = np.asarray(Wo, np.float32)
    bq = np.asarray(bq, np.float32)
    bk = np.asarray(bk, np.float32)
    bv = np.asarray(bv, np.float32)
    bo = np.asarray(bo, np.float32)

    in_maps = []
    for c in range(NCORES):
        sl = slice(c * DC, (c + 1) * DC)
        in_maps.append({
            "qT": qT, "kT": kT, "vT": vT,
            "wq": np.ascontiguousarray(Wq[:, sl]).astype(bf16),
            "wk": np.ascontiguousarray(Wk[:, sl]).astype(bf16),
            "wv": np.ascontiguousarray(Wv[:, sl]).astype(bf16),
            "wo": np.ascontiguousarray(Wo[sl, :]).astype(bf16),
            "bq": np.ascontiguousarray(bq[sl]).reshape(DC, 1),
            "bk": np.ascontiguousarray(bk[sl]).reshape(DC, 1),
        })

    nc = _get_program()
    res = run_bass_kernel_spmd(nc, in_maps, list(range(NCORES)), trace=trace)
    LAST_EXEC_NS = res.exec_time_ns
    LAST_RESULTS = res

    out = np.zeros((BS, D), np.float32)
    for c in range(NCORES):
        out += np.asarray(res.results[c]["out"], np.float32)
    out += bv.astype(np.float32) @ Wo + bo          # exact bias identities
    return out.reshape(B, S, D)
